# revision 20
# baseline (speedup 1.0000x reference)
"""Concat cost-volume kernel for Trainium2 (8 NeuronCores, SPMD).

Reference semantics (B=2, C=32, H=128, W=240, D=max_disp=48):
  out[b, c,      d, h, w] = left [b, c, h, w]     * (w >= d)   for c in [0, C)
  out[b, C + c,  d, h, w] = right[b, c, h, w - d] * (w >= d)   for c in [0, C)

This is pure data movement (~755 MB of output from ~16 MB of input), so the
kernel is HBM-write-bandwidth bound. Design:

* Sharding: channel-parallel. Core k builds the full disparity volume for
  channels [4k, 4k+4) of BOTH halves — all 8 cores run one identical program
  on different channel slices (no per-core compile specialization needed).

* Per-core dataflow:
  - left slice [2,4,128,240] -> SBUF [128(h), 8*240], one DMA.
  - right slice -> zero-padded SBUF tile [128(h), 8*288] (48 zero columns in
    front of each plane), one DMA. The shifted+masked right plane for
    disparity d is then just the sliding window cols [48-d : 288-d].
  - mask tile [128, 288] = 48 zeros then 240 ones; masked left plane for d is
    left * mask[48-d : 288-d].
  - staging: for each of the 16 (b, side, c) units, build the full
    [48, 240] x 128h volume in SBUF with D-fused DVE ops — a single
    tensor_mul/tensor_copy whose source AP has a broadcast (step 0) or
    sliding-window (step -1) disparity dimension covers a whole group of
    disparities in one instruction.
  - store: large multi-MB DMAs (HWDGE, alternating the two rings via nc.sync
    and nc.scalar). The output buffer is guaranteed pre-zeroed by
    run_bass_kernel_spmd (native path memsets, axon path donates zero
    buffers), so the all-zero wedge w < 16*floor(d/16) is never written
    ("staircase"), saving ~7% of write traffic.

Measured (loop-delta method on HW): ~277 us/core ~= the pure-DMA floor for
the same stores, ~337 GB/s of the ~358 GB/s per-core HBM write roofline.
"""

import dataclasses
import sys

import numpy as np

for _p in ("/opt/trn_rl_repo",):
    if _p not in sys.path:
        sys.path.insert(0, _p)

import concourse.bass as bass  # noqa: F401  (kept for interactive debugging)
import concourse.tile as tile
from concourse import bacc, mybir
from concourse.bass_utils import run_bass_kernel_spmd

B, C, H, W = 2, 32, 128, 240
D = 48
N_CORES = 8
CPC = C // N_CORES  # channels per core (per side) = 4
PAD = D  # zero-pad columns = 48
WPAD = W + PAD  # 288
NBC = B * CPC  # input planes per side per core = 8

# Tuned on hardware: 16-disparity staircase store groups, 16-disparity fused
# DVE staging ops. Matches the pure-DMA floor of the same store pattern.
BEST = dict(staircase=16, fused_staging=16)

_NC_CACHE = {}


def _build_nc(units=None, repeat=1, loop_n=None, skip_stores=False,
              skip_compute=False, dma_engines=("sync", "scalar"),
              staircase=0, copy_engine="vector", fused_staging=False,
              store_rr="unit"):
    """Build (and compile) the per-core SPMD program.

    Only `staircase`/`fused_staging`/`dma_engines` affect the production
    kernel; the other knobs exist for benchmarking variants (loop_n wraps the
    body in a hardware For_i for delta timing, skip_* isolate engines).
    """
    nc = bacc.Bacc("TRN2", target_bir_lowering=False, debug=False)
    f32 = mybir.dt.float32
    left_p = nc.declare_dram_parameter("left", [B, CPC, H, W], f32, isOutput=False)
    right_p = nc.declare_dram_parameter("right", [B, CPC, H, W], f32, isOutput=False)
    out_p = nc.declare_dram_parameter(
        "out", [B, 2 * CPC, D, H, W], f32, isOutput=True
    )
    if units is None:
        units = range(2 * NBC)

    with tile.TileContext(nc) as tc:
        with (
            tc.tile_pool(name="consts", bufs=1) as consts,
            tc.tile_pool(name="stage", bufs=3) as stagep,
        ):
            left_t = consts.tile([H, NBC * W], f32)
            right_t = consts.tile([H, NBC * WPAD], f32)
            mask_t = consts.tile([H, WPAD], f32)

            nc.vector.memset(mask_t[:, 0:PAD], 0.0)
            nc.vector.memset(mask_t[:, PAD:WPAD], 1.0)
            nc.gpsimd.memset(right_t[:, :], 0.0)

            nc.sync.dma_start(
                out=left_t[:, :].rearrange("h (k w) -> h k w", w=W),
                in_=left_p[:, :, :, :].rearrange("b c h w -> h (b c) w"),
            )
            nc.scalar.dma_start(
                out=right_t[:, :].rearrange("h (k w) -> h k w", w=WPAD)[:, :, PAD:],
                in_=right_p[:, :, :, :].rearrange("b c h w -> h (b c) w"),
            )

            engs = {"sync": nc.sync, "scalar": nc.scalar, "gpsimd": nc.gpsimd}
            const_st = None
            if skip_compute:
                const_st = consts.tile([H, D * W], f32, name="const_st")
                nc.vector.memset(const_st[:, :], 0.5)

            def stage_unit(st, side, bc):
                """Fill st ([128h, 48d * 240w]) with the unit's masked volume."""
                ceng = engs[copy_engine] if copy_engine != "vector" else nc.vector
                if fused_staging:
                    # One DVE op per group of gsz disparities: the source AP's
                    # middle dim walks the disparity axis via step 0 (left:
                    # same plane each d) or step -1 (right / mask: window
                    # start col 48-d slides left as d grows).
                    st3d = st[:, :].rearrange("h (d w) -> h d w", w=W)
                    gsz = D if fused_staging is True else int(fused_staging)
                    for g0 in range(0, D, gsz):
                        dst = st3d[:, g0 : g0 + gsz, :]
                        if side == 0:
                            lb = left_t[:, bc * W : (bc + 1) * W]
                            mb = mask_t[:, PAD - g0 : PAD - g0 + W]
                            nc.vector.tensor_mul(
                                dst,
                                dataclasses.replace(
                                    lb, ap=[lb.ap[0], [0, gsz], [1, W]]
                                ),
                                dataclasses.replace(
                                    mb, ap=[mb.ap[0], [-1, gsz], [1, W]]
                                ),
                            )
                        else:
                            rb = right_t[
                                :, bc * WPAD + PAD - g0 : bc * WPAD + PAD - g0 + W
                            ]
                            nc.vector.tensor_copy(
                                dst,
                                dataclasses.replace(
                                    rb, ap=[rb.ap[0], [-1, gsz], [1, W]]
                                ),
                            )
                else:
                    for d in range(D):
                        dst = st[:, d * W : (d + 1) * W]
                        if side == 0:
                            nc.vector.tensor_mul(
                                dst,
                                left_t[:, bc * W : (bc + 1) * W],
                                mask_t[:, PAD - d : WPAD - d],
                            )
                        else:
                            base = bc * WPAD
                            ceng.tensor_copy(
                                dst, right_t[:, base + PAD - d : base + WPAD - d]
                            )

            def store_unit(st, u, b, cc):
                eng = engs[dma_engines[u % len(dma_engines)]]
                if staircase:
                    # Rows d in group g are all-zero for w < g*G; those cells
                    # stay at the buffer's guaranteed zero fill and are
                    # skipped. Written region per group: w in [g*G, W).
                    G = staircase
                    st3 = st[:, :].rearrange("h (d w) -> h d w", w=W)
                    for g in range(D // G):
                        if store_rr == "group":
                            eng = engs[
                                dma_engines[(u * (D // G) + g) % len(dma_engines)]
                            ]
                        d0 = g * G
                        eng.dma_start(
                            out=out_p[b, cc, d0 : d0 + G, :, d0:]
                            .rearrange("d h w -> h d w"),
                            in_=st3[:, d0 : d0 + G, d0:],
                        )
                else:
                    eng.dma_start(
                        out=out_p[b, cc].rearrange("d h w -> h d w"),
                        in_=st[:, :],
                    )

            def body():
                for u in units:
                    b = u // (2 * CPC)
                    side = (u % (2 * CPC)) // CPC
                    c = u % CPC
                    bc = b * CPC + c
                    if skip_compute:
                        st = const_st
                    else:
                        st = stagep.tile([H, D * W], f32, tag="st", name="st")
                        stage_unit(st, side, bc)
                    if not skip_stores:
                        store_unit(st, u, b, side * CPC + c)

            if loop_n is not None:
                with tc.For_i(0, loop_n, 1):
                    body()
            else:
                for _rep in range(repeat):
                    body()
    nc.compile()
    return nc


def _get_nc():
    if "nc" not in _NC_CACHE:
        _NC_CACHE["nc"] = _build_nc(**BEST)
    return _NC_CACHE["nc"]


def _make_in_maps(left, right):
    in_maps = []
    for k in range(N_CORES):
        sl = slice(k * CPC, (k + 1) * CPC)
        in_maps.append(
            {
                "left": np.ascontiguousarray(left[:, sl]),
                "right": np.ascontiguousarray(right[:, sl]),
            }
        )
    return in_maps


def _assemble(results):
    out = np.empty((B, 2 * C, D, H, W), dtype=np.float32)
    for k in range(N_CORES):
        o = results[k]["out"]
        out[:, k * CPC : (k + 1) * CPC] = o[:, :CPC]
        out[:, C + k * CPC : C + (k + 1) * CPC] = o[:, CPC:]
    return out


def run(left_feature, right_feature, max_disp, **spmd_kwargs):
    """Run on hardware; returns (full_output, BassKernelResults)."""
    assert int(max_disp) == D
    left = np.ascontiguousarray(np.asarray(left_feature, dtype=np.float32))
    right = np.ascontiguousarray(np.asarray(right_feature, dtype=np.float32))
    assert left.shape == (B, C, H, W) and right.shape == (B, C, H, W)
    res = run_bass_kernel_spmd(
        _get_nc(), _make_in_maps(left, right), list(range(N_CORES)), **spmd_kwargs
    )
    return _assemble(res.results), res


def kernel(left_feature, right_feature, max_disp):
    out, _ = run(left_feature, right_feature, max_disp)
    return out
